# revision 37
# baseline (speedup 1.0000x reference)
"""Multi-head causal attention (RoPE) Trainium2 kernel, SPMD over 8 NeuronCores.

Sharding: core c handles batch b = c // 4 and head-group g = c % 4
(4 heads of 128 dims each => 512 output features per core). Fully
embarrassingly parallel - no collectives.

Schedule: a serial projections-then-attention kernel is ScalarE-bound
(softmax exp runs ~1.3ns/col, saturating ScalarE ~95us while the PE
idles), so heads are software-pipelined with cycle-proportional merged
emission:

  v-proj rows j0..7 (8 PSUM banks; DMA-paced against the input stream,
      which is split across the sync+gpsimd+scalar HWDGE rings - DMA
      issues block on ring credits, so bulk input must avoid queues
      whose engines later run latency-critical ops)
  merge[ v-proj rows j8..15 (2x4 banks) | qk-proj(head 0) ]
  early-emit scores chunks c0..c2 of head 0
  window h in 0..3:
    merge[ attention(h): c3 scores interleaved with PV rows + epilogues
         | qk-proj(h+1) if h < 3 ]
    then early-emit scores c0..c2 of head h+1 (their exps drain during
    the next window's projection stream).

ScalarE runs only exp (+4 v-evac copies); projection PSUM evacuation
(bias add) and RoPE mul/add are on VectorE; the RoPE half-swap is an
SBUF-SBUF DMA on the gpsimd ring; causal masks are applied in-place on
the exp tiles; V carries a ones column so the softmax denominator
accumulates for free in PSUM col 128. All matmuls bf16 with f32 PSUM.
"""

import sys

import numpy as np
import ml_dtypes

for _p in ("/opt/trn_rl_repo",):
    if _p not in sys.path:
        sys.path.insert(0, _p)

B, S, E = 2, 2048, 2048
H, D = 16, 128
P = 128
HPC = 4            # heads per core
F = HPC * D        # 512 projection features per core
NCORES = 8
NE = E // P        # 16 contraction tiles
NSQ = S // P       # 16 query row-tiles
NCH = S // 512     # 4 query chunks of 512
ROPE_BASE = 10000.0
SM_SCALE = 1.0 / float(np.sqrt(D))
BF16 = ml_dtypes.bfloat16

_compiled = None
LAST_RESULT = None

# interleaved (0,1),(2,3).. pairs -> half layout (i, i+64): new_i = old 2i,
# new_{i+64} = old 2i+1. Applied to q/k weight rows per head; scores are
# invariant since the same permutation hits q and k.
_PERM = np.concatenate([np.arange(0, D, 2), np.arange(1, D, 2)])


def _rope_tables():
    inv = ROPE_BASE ** (-np.arange(0, D, 2, dtype=np.float64) / D)      # [64]
    ang = np.arange(S, dtype=np.float64)[None, :] * inv[:, None]        # [64, S]
    cos, sin = np.cos(ang), np.sin(ang)
    cosf = np.concatenate([cos, cos], axis=0).astype(BF16)              # [128, S]
    ssin = np.concatenate([-sin, sin], axis=0).astype(BF16)
    return cosf, ssin


def _mask_tile():
    # mask[p, f] = 1 iff f >= p (lower-triangle-inclusive block)
    f = np.arange(512)[None, :]
    p = np.arange(P)[:, None]
    return (f >= p).astype(np.float32).astype(BF16)


def _merge(streams):
    """Cycle-proportional merge of emission streams.

    Each stream is (total_cycles, generator); each next() emits one work
    quantum and returns its PE-cycle estimate. Always advances the
    stream with the smallest fractional progress, so all streams finish
    emission together.
    """
    live = [[0.0, float(t), g] for t, g in streams if t > 0]
    while live:
        s = min(live, key=lambda x: x[0] / x[1])
        try:
            s[0] += next(s[2])
        except StopIteration:
            live.remove(s)


def _build():
    import concourse.mybir as mybir
    import concourse.tile as tile
    from concourse import bacc

    fdt = mybir.dt.float32
    bdt = mybir.dt.bfloat16
    Exp = mybir.ActivationFunctionType.Exp

    nc = bacc.Bacc("TRN2", target_bir_lowering=False, debug=False,
                   num_devices=NCORES)

    xt = nc.dram_tensor("xt", [E, S], bdt, kind="ExternalInput").ap()
    wqt = nc.dram_tensor("wqt", [E, F], bdt, kind="ExternalInput").ap()
    wkt = nc.dram_tensor("wkt", [E, F], bdt, kind="ExternalInput").ap()
    wvt = nc.dram_tensor("wvt", [E, F], bdt, kind="ExternalInput").ap()
    bqd = nc.dram_tensor("bqd", [P, HPC], fdt, kind="ExternalInput").ap()
    bkd = nc.dram_tensor("bkd", [P, HPC], fdt, kind="ExternalInput").ap()
    bvbd = nc.dram_tensor("bvbd", [P, F], fdt, kind="ExternalInput").ap()
    cosd = nc.dram_tensor("cosd", [P, S], bdt, kind="ExternalInput").ap()
    ssind = nc.dram_tensor("ssind", [P, S], bdt, kind="ExternalInput").ap()
    maskd = nc.dram_tensor("maskd", [P, 512], bdt, kind="ExternalInput").ap()
    outd = nc.dram_tensor("out", [S, F], fdt, kind="ExternalOutput").ap()

    PSTAGS = ["pp0", "pp1", "sc0", "sc1", "sc2", "sc3", "pv0", "pv1"]

    with tile.TileContext(nc) as tc:
        with (
            tc.tile_pool(name="const", bufs=1) as constp,
            tc.tile_pool(name="persist", bufs=1) as persist,
            tc.tile_pool(name="psum", bufs=1, space="PSUM") as psp,
        ):
            # biases are tiny and needed mid-kernel; the big rope/mask/bias
            # constants are emitted AFTER the weight stream (below) so the
            # scalar HWDGE queue can carry half the input bandwidth first.
            bqpt = constp.tile([P, HPC], fdt, tag="bqpt", name="bqpt")
            nc.scalar.dma_start(bqpt[:], bqd[:])
            bkpt = constp.tile([P, HPC], fdt, tag="bkpt", name="bkpt")
            nc.scalar.dma_start(bkpt[:], bkd[:])
            cos_sb = constp.tile([P, S], bdt, tag="cos", name="cos_sb")
            ssin_sb = constp.tile([P, S], bdt, tag="ssin", name="ssin_sb")
            mask_sb = constp.tile([P, 512], bdt, tag="mask", name="mask_sb")
            bvb_sb = constp.tile([P, F], fdt, tag="bvb", name="bvb_sb")

            # persistent SBUF: x^T tiles, q/k weights, q^T/k^T (2-head
            # rotation), V-with-ones tiles.
            xts = [persist.tile([P, S], bdt, tag=f"x{e}", name=f"x{e}")
                   for e in range(NE)]
            wqs = [persist.tile([P, F], bdt, tag=f"wq{e}", name=f"wq{e}")
                   for e in range(NE)]
            wks = [persist.tile([P, F], bdt, tag=f"wk{e}", name=f"wk{e}")
                   for e in range(NE)]
            qTt = [persist.tile([P, S], bdt, tag=f"qT{p}", name=f"qT{p}")
                   for p in range(2)]
            kTt = [persist.tile([P, S], bdt, tag=f"kT{p}", name=f"kT{p}")
                   for p in range(2)]
            qT = lambda h: qTt[h % 2]
            kT = lambda h: kTt[h % 2]
            # vA[j]: [128, 4, 129] = per head 128 v-cols + a ones column (the
            # softmax denominator accumulates for free in PSUM col 128).
            vA = [persist.tile([P, HPC, D + 1], bdt, tag=f"vA{j}",
                               name=f"vA{j}") for j in range(NSQ)]

            # ---------------- v projection ------------------------------
            # v1 (rows j0..7) consumes only the first xt half-tiles, so the
            # DMA stream (wv_e, xt_e lo-half) feeds it at matched rate; the
            # second halves / q / k weights follow on the sync queue. v2 is
            # emitted later, merged with qk-proj(0).
            # Effective DMA bandwidth is ~96GB/s PER RING early on and issues
            # block on ring credits, so bulk input is spread over THREE idle
            # engine queues (sync/gpsimd/vector). NOT the scalar queue: exp
            # and evac ops queued behind bulk DMAs would stall ~40us. The
            # vector queue's share is only the early v1 pairs so it drains
            # before the first evacuation copies. wq goes ahead of the xt
            # high halves: qk-proj(0) needs it before v2 finishes streaming.
            # the scalar ring carries every third pair (~1.9MB drains by
            # ~23us, clear of the first evac copies / exps that share its
            # engine queue) so v1's three-ring feed matches its ~220GB/s
            # streaming demand.
            wvs = []
            Q = [nc.sync, nc.gpsimd]
            QP = ([nc.sync, nc.gpsimd, nc.scalar] * 6)[:NE]
            for e in range(NE):
                wtile = persist.tile([P, F], bdt, tag=f"wv{e}", name=f"wv{e}")
                QP[e].dma_start(wtile[:], wvt[P * e:P * (e + 1), :])
                wvs.append(wtile)
                if e < 3:
                    # dep tracking is region-based: splitting lets the first
                    # matmuls start ~2us sooner during the DMA-rate warmup
                    QP[e].dma_start(xts[e][:, 0:512],
                                    xt[P * e:P * (e + 1), 0:512])
                    QP[e].dma_start(xts[e][:, 512:1024],
                                    xt[P * e:P * (e + 1), 512:1024])
                else:
                    QP[e].dma_start(xts[e][:, 0:1024],
                                    xt[P * e:P * (e + 1), 0:1024])
            for e in range(NE):
                Q[e % 2].dma_start(wqs[e][:], wqt[P * e:P * (e + 1), :])
                Q[e % 2].dma_start(xts[e][:, 1024:2048],
                                   xt[P * e:P * (e + 1), 1024:2048])
            # big constants trail the weight stream on the sync/gpsimd rings
            # (NOT scalar: its ring must drain before the v1 evac copies).
            # First needed: cos at the head-0 rope (~45us), mask at the first
            # diagonal exp (~65us), bvb at the first PV epilogue (~75us).
            nc.gpsimd.dma_start(cos_sb[:], cosd[:])
            nc.gpsimd.dma_start(ssin_sb[:], ssind[:])
            nc.sync.dma_start(mask_sb[:], maskd[:])
            nc.sync.dma_start(bvb_sb[:], bvbd[:])
            def v_evac(pss, jj, eng):
                if eng is nc.scalar:
                    eng.copy(vA[jj][:, :, 0:D], pss[:, :, :])
                else:
                    eng.tensor_copy(vA[jj][:, :, 0:D], pss[:, :, :])

            # v1: 8 PSUM banks, rows j0..7. The ones-columns have no deps, so
            # they are set up front; evacuations are interleaved into the
            # final accumulation round, split across DVE and ScalarE, ordered
            # so the merged (v2, qk-proj0) stream's PSUM reuse unblocks first.
            for j in range(NSQ):
                nc.vector.memset(vA[j][:, :, D:D + 1], 1.0)
            pss = [psp.tile([P, HPC, D], fdt, tag=PSTAGS[j], name=f"vps{j}")
                   for j in range(8)]
            for e in range(NE - 1):
                for j in range(8):
                    nc.tensor.matmul(
                        pss[j][:], xts[e][:, P * j:P * (j + 1)],
                        wvs[e][:], start=(e == 0), stop=False)
            e = NE - 1
            for i, j in enumerate((2, 3, 4, 5, 0, 1, 6, 7)):
                nc.tensor.matmul(
                    pss[j][:], xts[e][:, P * j:P * (j + 1)],
                    wvs[e][:], start=False, stop=True)
                v_evac(pss[j], j, nc.vector if i % 2 == 0 else nc.scalar)
            # wk rides the scalar ring BEHIND the evac copies (in engine
            # order), off-loading 2MB from the sync/gpsimd rings whose xt
            # hi-half delivery paces the merged v2/qk0 stream; wk_e lands
            # ~41-60us, just ahead of the k-projection passes (~52-62us).
            for e in range(NE):
                nc.scalar.dma_start(wks[e][:], wkt[P * e:P * (e + 1), :])

            def gen_v2():
                # rows j8..15 in two sub-passes on the sc0..3 banks (pp/pv
                # tags stay free for the merged qk-proj(0) stream).
                for jbase in (8, 12):
                    pss = [psp.tile([P, HPC, D], fdt, tag=f"sc{j}",
                                    name=f"vps{jbase + j}")
                           for j in range(4)]
                    for e in range(NE):
                        for jp in (0, 2):
                            for j in (jp, jp + 1):
                                jj = jbase + j
                                nc.tensor.matmul(
                                    pss[j][:], xts[e][:, P * jj:P * (jj + 1)],
                                    wvs[e][:], start=(e == 0),
                                    stop=(e == NE - 1))
                            yield 1024
                    for j in range(4):
                        v_evac(pss[j], jbase + j, nc.vector)
                    yield 100

            with (
                tc.tile_pool(name="et01", bufs=2) as etp01,
                tc.tile_pool(name="et23", bufs=1) as etp23,
                tc.tile_pool(name="rope", bufs=2) as ropep,
                tc.tile_pool(name="ost", bufs=3) as osp,
            ):
                # -------- emission streams --------
                def gen_proj(h):
                    """qk projection of head h: 2 projections x 2 chunk-pair
                    passes; RoPE applied during PSUM evacuation."""
                    for wts, bias_pt, dstf in (
                        (wqs, bqpt, qT), (wks, bkpt, kT),
                    ):
                        dst = dstf(h)
                        pfx = "q" if wts is wqs else "k"
                        for pp in (0, 1):
                            pss = [psp.tile([P, 512], fdt, tag=f"pp{i}",
                                            name=f"{pfx}ps{h}_{2 * pp + i}")
                                   for i in range(2)]
                            for e in range(NE):
                                for i in range(2):
                                    c = 2 * pp + i
                                    nc.tensor.matmul(
                                        pss[i][:],
                                        wts[e][:, P * h:P * (h + 1)],
                                        xts[e][:, 512 * c:512 * (c + 1)],
                                        start=(e == 0), stop=(e == NE - 1))
                                yield 1024
                            for i in range(2):
                                c = 2 * pp + i
                                cs = slice(512 * c, 512 * (c + 1))
                                xs = ropep.tile([P, 512], bdt, tag="xs",
                                                name=f"{pfx}xs{h}_{c}")
                                nc.vector.tensor_scalar_add(
                                    xs[:], pss[i][:], bias_pt[:, h:h + 1])
                                sw = ropep.tile([P, 512], bdt, tag="sw",
                                                name=f"{pfx}sw{h}_{c}")
                                nc.gpsimd.dma_start(sw[0:64, :], xs[64:128, :])
                                nc.gpsimd.dma_start(sw[64:128, :], xs[0:64, :])
                                nc.vector.tensor_mul(xs[:], xs[:],
                                                     cos_sb[:, cs])
                                nc.vector.tensor_mul(sw[:], sw[:],
                                                     ssin_sb[:, cs])
                                nc.vector.tensor_add(dst[:, cs], xs[:], sw[:])
                                yield 100

                ets = {}   # (h%2 irrelevant: live per head) -> et tiles

                sc_seq = [0]

                def emit_score_tile(h, c, t):
                    """scores^T tile [k=128, w] for key-tile t, q-chunk c,
                    exp'ed (and causal-masked in place if diagonal)."""
                    diag = t >= 4 * c
                    o = P * (t % 4) if diag else 0
                    w = 512 - o
                    cs = slice(512 * c + o, 512 * (c + 1))
                    i = sc_seq[0] % 4
                    sc_seq[0] += 1
                    ps_sc = psp.tile([P, w], fdt, tag=f"sc{i}",
                                     name=f"sc{h}_{c}_{t}")
                    nc.tensor.matmul(
                        ps_sc[:], kT(h)[:, P * t:P * (t + 1)],
                        qT(h)[:, cs], start=True, stop=True)
                    pool = etp01 if c < 2 else etp23
                    et = pool.tile([P, w], bdt, tag=f"et{c}_{t}",
                                   name=f"et{h}_{c}_{t}")
                    nc.scalar.activation(et[:], ps_sc[:], Exp, scale=SM_SCALE)
                    if diag:
                        nc.vector.tensor_mul(et[:], et[:], mask_sb[:, 0:w])
                    ets[(h, c, t)] = (et, o)
                    return w

                def emit_scores_chunk(h, c):
                    # first diag tile, then bulk, then remaining diag tiles:
                    # PV row 4c+m becomes ready after queue position 4c+1+m,
                    # so rows unblock incrementally as exps drain.
                    order = ([4 * c] + list(range(0, 4 * c))
                             + list(range(4 * c + 1, 4 * c + 4)))
                    for t in order:
                        yield emit_score_tile(h, c, t)

                def emit_pv_row(h, j):
                    po = psp.tile([P, D + 1], fdt, tag=f"pv{j % 2}",
                                  name=f"po{h}_{j}")
                    jj = j % 4
                    for t in range(j + 1):
                        et, o = ets[(h, j // 4, t)]
                        lo = P * jj - o
                        nc.tensor.matmul(
                            po[:], et[:, lo:lo + P], vA[t][:, h, :],
                            start=(t == 0), stop=(t == j))
                    rec = osp.tile([P, 1], fdt, tag="rec", name=f"rec{h}_{j}")
                    nc.vector.reciprocal(rec[:], po[:, D:D + 1])
                    ot = osp.tile([P, D], fdt, tag="ot", name=f"ot{h}_{j}")
                    nc.vector.scalar_tensor_tensor(
                        ot[:], po[:, 0:D], rec[:],
                        bvb_sb[:, D * h:D * (h + 1)],
                        mybir.AluOpType.mult, mybir.AluOpType.add)
                    nc.sync.dma_start(
                        outd[P * j:P * (j + 1), D * h:D * (h + 1)], ot[:])
                    return (j + 1) * (D + 1) + 80

                def gen_attn(h):
                    """attention for head h, minus the c0/c1/c2 score tiles
                    (emitted early at the end of the previous window). The
                    first 8 PV rows (deps long ready) interleave with the
                    c3 score tiles so their exps get a queue head start."""
                    j = 0
                    for i, w in enumerate(emit_scores_chunk(h, 3)):
                        yield w
                        if i % 2 == 1:
                            yield emit_pv_row(h, j)
                            j += 1
                    # final head ends on a short row so the last epilogue +
                    # out-DMA chain overlaps PV matmuls instead of trailing
                    tail = ([13, 14, 15, 12] if h == HPC - 1
                            else [12, 13, 14, 15])
                    for j in list(range(j, 12)) + tail:
                        yield emit_pv_row(h, j)

                def emit_early_scores(h):
                    for c in (0, 1, 2):
                        for _ in emit_scores_chunk(h, c):
                            pass

                ATT_CYC = 7424 + sum(
                    (j + 1) * (D + 1) + 80 for j in range(NSQ))
                PROJ_CYC = 2 * 2 * (NE * 1024 + 2 * 100)
                V2_CYC = 2 * (NE * 2048 + 100)

                # v2 merged with qk-proj(0), then early scores for head 0
                _merge([(V2_CYC, gen_v2()), (PROJ_CYC, gen_proj(0))])
                emit_early_scores(0)

                for h in range(HPC):
                    streams = [(ATT_CYC, gen_attn(h))]
                    if h + 1 < HPC:
                        streams.append((PROJ_CYC, gen_proj(h + 1)))
                    _merge(streams)
                    if h + 1 < HPC:
                        emit_early_scores(h + 1)

    nc.compile()
    return nc


def get_compiled():
    global _compiled
    if _compiled is None:
        _compiled = _build()
    return _compiled


def make_in_maps(logits, Wq, bq, Wk, bk, Wv, bv):
    cosf, ssin = _rope_tables()
    maskm = _mask_tile()
    xts = [np.ascontiguousarray(np.asarray(logits)[b].T).astype(BF16)
           for b in range(B)]

    def permW(Wm, rows):
        Wp = np.asarray(Wm)[rows].reshape(HPC, D, E)[:, _PERM, :].reshape(F, E)
        return np.ascontiguousarray(Wp.T).astype(BF16)

    def permb(bvec, rows):
        # [128, HPC] f32: column h = permuted bias of head h
        return np.ascontiguousarray(
            np.asarray(bvec)[rows].reshape(HPC, D)[:, _PERM].T
        ).astype(np.float32)

    in_maps = []
    for core in range(NCORES):
        b, g = divmod(core, 4)
        rows = slice(F * g, F * (g + 1))
        in_maps.append({
            "xt": xts[b],
            "wqt": permW(Wq, rows),
            "wkt": permW(Wk, rows),
            "wvt": np.ascontiguousarray(np.asarray(Wv)[rows].T).astype(BF16),
            "bqd": permb(bq, rows),
            "bkd": permb(bk, rows),
            "bvbd": np.ascontiguousarray(np.broadcast_to(
                np.asarray(bv)[rows].astype(np.float32), (P, F))),
            "cosd": cosf,
            "ssind": ssin,
            "maskd": maskm,
        })
    return in_maps


def kernel(logits, Wq, bq, Wk, bk, Wv, bv, **_ignored):
    global LAST_RESULT
    from concourse.bass_utils import run_bass_kernel_spmd

    nc = get_compiled()
    in_maps = make_in_maps(logits, Wq, bq, Wk, bk, Wv, bv)
    res = run_bass_kernel_spmd(nc, in_maps, list(range(NCORES)))
    LAST_RESULT = res
    out = np.empty((B, S, H * D), dtype=np.float32)
    for core in range(NCORES):
        b, g = divmod(core, 4)
        out[b, :, F * g:F * (g + 1)] = res.results[core]["out"]
    return out
